# revision 26
# baseline (speedup 1.0000x reference)
"""Causal single-head attention (B=4, S=4096, E=32, H=64) on 8 TRN2 NeuronCores.

Sharding: core c handles batch b=c//2 and query parity p=c%2 (query chunks of
256 rows, chunks p, p+2, ..., p+14 of that batch). Causal work per chunk grows
linearly, so parity interleaving balances the load while keeping control flow
identical on every core (one SPMD NEFF); all per-core differences are input
data (host-permuted queries and host-built masks).

Device algorithm per core (slot s = 0..7, 256 queries each, E_s = 4s+4 key
chunks of 128):
  QT/KT = [W;b].T @ [x^T;1]  (bias folded into the matmul via ones row)
  V     = [x^T;1].T @ [Wv,0;bv,1]  (ones column appended -> denominator)
  ST[j,q] = KT_chunk.T @ QT_slot   (scores transposed, contraction = H)
  PT = exp(ST/8)  (ACT, scale folded into activation; no max subtraction --
                   |scores/8| < ~2 for this distribution)
  PT *= mask      (only last 4 chunks of each slot are not fully causal)
  ACC[h,q] += V_chunk.T @ PT       (PSUM accumulation over chunks)
  out[q,h] = transpose(ACC)[:, :64] / transpose(ACC)[:, 64]
"""

import numpy as np
import ml_dtypes

B, S, E, H = 4, 4096, 32, 64
P = 128
NQ = S // 2          # queries per core
SLOTS = 8            # 256-query slots per core
SQ = 256             # queries per slot

_BF16 = ml_dtypes.bfloat16

_cache = {}


def _mark(nc, label):
    """Record (label, #instructions) build marks for sim-profiling."""
    marks = getattr(nc, "_phase_marks", None)
    if marks is None:
        marks = []
        nc._phase_marks = marks
    marks.append((label, len(nc.inst_map)))


def _build_bass(
    reps=1,
    scb=4,            # key chunks per score-psum batch (one ACT call each)
    score_bufs=3,
    pt_bufs=3,
    mask_engine="vector",   # vector | gpsimd
    proj_split=False,       # route first proj psum->sbuf copies to ACT
    slot_order=None,
    skip_exp=False,         # timing probe: omit exp+mask+PV+fin
    skip_qk=False,          # timing probe: omit QK too (with skip_exp)
    skip_pv=False,          # timing probe: omit PV+fin only
    skip_mask=False,        # timing probe: omit mask multiplies
    qk_pack=True,           # row-pack QK pairs into 64-row array halves
):
    skip_pv = skip_pv or skip_exp
    skip_mask = skip_mask or skip_exp
    """Build the SPMD NEFF. reps>1 repeats the whole computation (with a
    scheduling barrier between reps) -- used only for wall-clock timing."""
    import concourse.tile as tile
    from concourse import bacc, mybir
    from concourse.masks import make_identity

    f32 = mybir.dt.float32
    bf16 = mybir.dt.bfloat16
    Exp = mybir.ActivationFunctionType.Exp
    SCALE = 1.0 / float(np.sqrt(H))

    nc = bacc.Bacc(None, target_bir_lowering=False)
    xqT = nc.dram_tensor("xqT", [E + 1, NQ], bf16, kind="ExternalInput")
    xkvT = nc.dram_tensor("xkvT", [E + 1, S], bf16, kind="ExternalInput")
    wqkv = nc.dram_tensor("wqkv", [E + 1, 2 * H + H + 1], bf16,
                          kind="ExternalInput")
    mask4 = nc.dram_tensor("mask4", [P, 4, SQ], bf16, kind="ExternalInput")
    out = nc.dram_tensor("out", [NQ, H], f32, kind="ExternalOutput")

    NKC = S // P  # 32 key chunks
    if slot_order is None:
        slot_order = list(range(SLOTS))

    with tile.TileContext(nc) as tc:
        with (
            tc.tile_pool(name="const", bufs=1) as cpool,
            tc.tile_pool(name="pt", bufs=pt_bufs) as ptpool,
            tc.tile_pool(name="fin", bufs=2) as finpool,
            tc.tile_pool(name="ps_score", bufs=score_bufs, space="PSUM") as spool,
            tc.tile_pool(name="ps_pv", bufs=1, space="PSUM") as pvpool,
            tc.tile_pool(name="ps_tr", bufs=1, space="PSUM") as trpool,
        ):
            for rep in range(reps):
                if rep:
                    tc.strict_bb_all_engine_barrier()
                # ---- load inputs. Each DMA costs ~625ns of serialized HWDGE
                #      queue time regardless of size -> few, big DMAs, in
                #      dependency-criticality order.
                w_sb = cpool.tile([E + 1, 2 * H + H + 1], bf16, tag="w")
                nc.sync.dma_start(w_sb[:], wqkv[:])
                xq_sb = cpool.tile([E + 1, NQ], bf16, tag="xq")
                nc.sync.dma_start(xq_sb[:], xqT[:])
                wq_sb = w_sb[:, 0:H]
                wk_sb = w_sb[:, H:2 * H]
                wv_sb = w_sb[:, 2 * H:2 * H + H + 1]
                xkv_sb = cpool.tile([E + 1, S], bf16, tag="xkv")
                nc.sync.dma_start(xkv_sb[:], xkvT[:])
                mask_sb = cpool.tile([P, 4, SQ], bf16, tag="mask")
                nc.sync.dma_start(mask_sb[:], mask4[:])
                xq_t = [xq_sb[:, c * 512:(c + 1) * 512]
                        for c in range(NQ // 512)]
                xkv_t = [xkv_sb[:, c * 512:(c + 1) * 512]
                         for c in range(S // 512)]
                ident = cpool.tile([P, P], f32, tag="ident")
                make_identity(nc, ident[:])

                _mark(nc, "load")

                def proj_copy(idx, dst, src):
                    if proj_split and idx < 3:
                        nc.scalar.copy(dst, src)
                    else:
                        nc.vector.tensor_copy(dst, src)

                # ---- projections are emitted interleaved with the slot
                #      loop (PE runs its queue in order; emitting all
                #      projections first would stall attention ~10us).
                QKP = P if qk_pack else H
                qt_t = [cpool.tile([QKP, 512], bf16, tag=f"qt{c}", name=f"qt{c}")
                        for c in range(NQ // 512)]
                kt_t = [cpool.tile([QKP, 512], bf16, tag=f"kt{c}", name=f"kt{c}")
                        for c in range(S // 512)]
                v_t = [cpool.tile([P, 4, H + 1], bf16, tag=f"v{g}", name=f"v{g}")
                       for g in range(NKC // 4)]

                def proj_qk_mm(ps, w, x):
                    # With qk_pack, produce [w.T@x; w.T@x] stacked on
                    # partitions via two col-packed concurrent matmuls.
                    nc.tensor.matmul(
                        ps[:H, :], w, x, start=True, stop=True,
                    )
                    if qk_pack:
                        nc.tensor.matmul(
                            ps[H:2 * H, :], w, x, start=True, stop=True,
                            tile_position=(0, H),
                        )

                def emit_proj(c):
                    if c < NQ // 512:
                        ps = spool.tile([QKP, 512], f32, tag="score",
                                        name=f"ps_q{c}")
                        proj_qk_mm(ps, wq_sb, xq_t[c])
                        proj_copy(c, qt_t[c][:], ps[:])
                    ps = spool.tile([QKP, 512], f32, tag="score", name=f"ps_k{c}")
                    proj_qk_mm(ps, wk_sb, xkv_t[c])
                    proj_copy(c + 1, kt_t[c][:], ps[:])
                    ps = spool.tile([P, 4, H + 1], f32, tag="score",
                                    name=f"ps_v{c}")
                    for i in range(4):
                        nc.tensor.matmul(
                            ps[:, i, :], xkv_t[c][:, i * P:(i + 1) * P], wv_sb,
                            start=True, stop=True,
                        )
                    proj_copy(c, v_t[c][:], ps[:])

                _mark(nc, "proj")
                # ---- main attention loop (proj chunk s emitted just
                #      before slot s; slot s depends on chunks 0..s) ----
                emitted_proj = set()

                def ensure_proj(upto):
                    for c in range(upto + 1):
                        if c not in emitted_proj:
                            emitted_proj.add(c)
                            emit_proj(c)

                def emit_fin(s, acc_ps):
                    acc_sb = finpool.tile([H + 1, SQ], f32, tag="acc_sb",
                                          name=f"acc_sb{s}")
                    nc.vector.tensor_copy(acc_sb[:], acc_ps[:])
                    for hh in range(2):
                        tr_ps = trpool.tile([P, H + 1], f32, tag="tr",
                                            name=f"tr{s}_{hh}")
                        nc.tensor.transpose(
                            tr_ps[:], acc_sb[:, hh * P:(hh + 1) * P],
                            ident[: H + 1, : H + 1],
                        )
                        rec = finpool.tile([P, 1], f32, tag="rec",
                                           name=f"rec{s}_{hh}")
                        nc.vector.reciprocal(rec[:], tr_ps[:, H:H + 1])
                        o_sb = finpool.tile([P, H], f32, tag="o",
                                            name=f"o{s}_{hh}")
                        nc.vector.tensor_scalar_mul(o_sb[:], tr_ps[:, :H],
                                                    rec[:])
                        r0 = s * SQ + hh * P
                        nc.sync.dma_start(out[r0:r0 + P, :], o_sb[:])
                    _mark(nc, f"slot{s}_fin")

                def emit_pv(s, b0, nb, ext, pt_sb, acc_ps):
                    if skip_pv:
                        return
                    for i in range(nb):
                        jc = b0 + i
                        nc.tensor.matmul(
                            acc_ps[:], v_t[jc // 4][:, jc % 4, :],
                            pt_sb[:, i, :],
                            start=(jc == 0), stop=(jc == ext - 1),
                            skip_group_check=True,
                        )
                    if b0 + nb == ext:
                        _mark(nc, f"slot{s}_main")
                        emit_fin(s, acc_ps)

                # Flat software pipeline over all (slot, batch) items with a
                # one-batch emission lookahead: PE's FIFO sees QK(k+1) before
                # PV(k), so ACT's exp stream never stalls at slot boundaries.
                batches = []
                for s in slot_order:
                    ext = 4 * s + 4
                    for b0 in range(0, ext, scb):
                        batches.append((s, b0, min(scb, ext - b0), ext))

                acc_of = {}
                pending = None  # (s, b0, nb, ext, pt_sb, acc_ps)
                for (s, b0, nb, ext) in batches:
                    if b0 == 0:
                        ensure_proj(s)
                        if not skip_pv:
                            acc_of[s] = pvpool.tile(
                                [H + 1, SQ], f32, tag="acc", name=f"acc{s}")
                        else:
                            acc_of[s] = None
                    qs = qt_t[s // 2][:, (s % 2) * SQ:(s % 2 + 1) * SQ]
                    st_ps = spool.tile([P, scb, SQ], f32, tag="score",
                                       name=f"st{s}_{b0}")
                    if qk_pack and not skip_qk and nb == 4:
                        # pairs (0,2) and (1,3): the packed partners write
                        # different PSUM banks and use different array halves
                        for a in (0, 1):
                            for half, i in ((0, a), (1, a + 2)):
                                jc = b0 + i
                                kts = kt_t[jc // 4][
                                    half * H:(half + 1) * H,
                                    (jc % 4) * P:(jc % 4 + 1) * P,
                                ]
                                nc.tensor.matmul(
                                    st_ps[:, i, :], kts,
                                    qs[half * H:(half + 1) * H, :],
                                    start=True, stop=True,
                                )
                    else:
                        for i in range(nb):
                            if skip_qk:
                                break
                            jc = b0 + i
                            nc.tensor.matmul(
                                st_ps[:, i, :],
                                kt_t[jc // 4][:H, (jc % 4) * P:(jc % 4 + 1) * P],
                                qs[:H, :],
                                start=True, stop=True,
                            )
                    if pending is not None:
                        emit_pv(*pending)
                    pt_sb = ptpool.tile([P, scb, SQ], bf16, tag="pt",
                                        name=f"pt{s}_{b0}")
                    if not skip_exp:
                        nc.scalar.activation(
                            pt_sb[:, :nb, :], st_ps[:, :nb, :], Exp, scale=SCALE,
                        )
                    for i in range(nb):
                        jc = b0 + i
                        k = jc - (ext - 4)
                        if k >= 0 and not skip_mask:
                            eng = (
                                nc.gpsimd if mask_engine == "gpsimd"
                                else nc.vector
                            )
                            eng.tensor_mul(
                                pt_sb[:, i, :], pt_sb[:, i, :],
                                mask_sb[:, k, :],
                            )
                    pending = (s, b0, nb, ext, pt_sb, acc_of[s])
                if pending is not None:
                    emit_pv(*pending)

    nc.compile()
    return nc


def _host_inputs(x, Wq, bq, Wk, bk, Wv, bv):
    """Build the 8 per-core input maps."""
    ones_q = np.ones((1, NQ), np.float32)
    ones_s = np.ones((1, S), np.float32)
    wq_in = np.concatenate([Wq, bq[None, :]], axis=0)
    wk_in = np.concatenate([Wk, bk[None, :]], axis=0)
    wv_full = np.zeros((E + 1, H + 1), np.float32)
    wv_full[:E, :H] = Wv
    wv_full[E, :H] = bv
    wv_full[E, H] = 1.0
    wqkv_in = np.concatenate([wq_in, wk_in, wv_full], axis=1).astype(_BF16)

    r = np.arange(P)[:, None]
    f = np.arange(SQ)[None, :]
    m0 = (r <= f).astype(np.float32)
    m1 = (r + P <= f).astype(np.float32)
    zz = np.zeros((P, SQ), np.float32)
    oo = np.ones((P, SQ), np.float32)
    masks = [
        np.stack([m0, m1, zz, zz]).astype(_BF16),  # parity 0
        np.stack([oo, oo, m0, m1]).astype(_BF16),  # parity 1
    ]

    in_maps = []
    for c in range(8):
        b, p = divmod(c, 2)
        xb = x[b]  # [S, E]
        rows = np.concatenate(
            [np.arange(u * SQ, (u + 1) * SQ) for u in range(p, 16, 2)]
        )
        xq = xb[rows]  # [NQ, E]
        xqT = np.concatenate([xq.T, ones_q], axis=0).astype(_BF16)
        xkvT = np.concatenate([xb.T, ones_s], axis=0).astype(_BF16)
        in_maps.append({
            "xqT": np.ascontiguousarray(xqT),
            "xkvT": np.ascontiguousarray(xkvT),
            "wqkv": wqkv_in,
            "mask4": masks[p].transpose(1, 0, 2).copy(),  # [P, 4, SQ]
        })
    return in_maps


def _unshard(results):
    out = np.empty((B, S, H), np.float32)
    for c in range(8):
        b, p = divmod(c, 2)
        oc = results[c]["out"]  # [NQ, H]
        for si, u in enumerate(range(p, 16, 2)):
            out[b, u * SQ:(u + 1) * SQ, :] = oc[si * SQ:(si + 1) * SQ, :]
    return out


def kernel(x, Wq, bq, Wk, bk, Wv, bv):
    from concourse.bass_utils import run_bass_kernel_spmd

    x = np.asarray(x, np.float32)
    Wq = np.asarray(Wq, np.float32)
    bq = np.asarray(bq, np.float32)
    Wk = np.asarray(Wk, np.float32)
    bk = np.asarray(bk, np.float32)
    Wv = np.asarray(Wv, np.float32)
    bv = np.asarray(bv, np.float32)

    if "nc" not in _cache:
        _cache["nc"] = _build_bass()
    nc = _cache["nc"]

    in_maps = _host_inputs(x, Wq, bq, Wk, bk, Wv, bv)
    res = run_bass_kernel_spmd(nc, in_maps, core_ids=list(range(8)))
    return _unshard(res.results)
